# revision 4
# baseline (speedup 1.0000x reference)
"""Trainium2 Bass kernel for NeighborsValuesAssigner (retrieval_knn).

out[b,:,h,w] = mean_{n in top8} values[n]  where top8 = 8 largest
score[b,n,h,w] = <p_n, x_patch(b,h,w)> - 0.5||p_n||^2  (5x5 'same' conv).

8 cores, data-parallel over batch (4 images/core). Per core, two passes
over a hi/lo fp16-split score matmul (error ~2^-22, below fp32 rounding):

  phase A  score[px, n] on PE (stationary = x-patch tile [76,128],
           moving = patch dict) -> PSUM [128,1024] halves; DVE max8 per
           half + merge -> per-pixel 8th-largest threshold t8 (fp32).
  boundary t8 -> hi/lo fp16 split (DVE) -> PE transpose (f32) ->
           ACT copy -> SBUF [tile, px] -> 2 DMAs write t8h/t8l as row 76
           of the big x tiles.
  phase B  score'[n, px] = score - t8 recomputed TRANSPOSED with the
           same three hi/lo product terms in the same accumulation order
           (bit-consistent modulo ~1 ulp) with -t8h/-t8l folded in as
           contraction row 76; DVE is_ge(score', -1e-4) -> {0,1} fp16
           mask [n, px]; PE accumulates out[D, px] += (values/8)^T @ mask
           over 16 n-chunks; PSUM -> DRAM DMA directly.

No PE transposes of masks, no mask-copy ACT traffic: the PE stream is
pure back-to-back matmuls, which also keeps the HAM clock gate at 2.4 GHz.
"""
import sys

sys.path.insert(0, "/opt/trn_rl_repo")

import numpy as np
import ml_dtypes

B, C, H, W = 32, 3, 64, 64
N, D = 2048, 128
KH = KW = 5
KDIM = C * KH * KW          # 75
KROWS = KDIM + 2            # 77 = patch dims + bias row + threshold row
BIAS_ROW = KDIM             # 75
THR_ROW = KDIM + 1          # 76
NCORES = 8
BLOC = B // NCORES          # 4 images per core
PX = BLOC * H * W           # 16384 pixels per core
NTILE = PX // 128           # 128 pixel-tiles per core (phase A)
GPX = 512                   # pixels per group (phase B)
NGRP = PX // GPX            # 32 groups per core
NCHUNK = N // 128           # 16 patch chunks
EPS = 1.0e-4                # is_ge margin: >> recompute noise (~3e-5),
                            # << typical 8th/9th score gap (~0.5)

_CACHE = {}


def _build_program(loop_r=0):
    """loop_r=0: straight-line. loop_r>0: wrap body in a device-side
    For_i loop running it loop_r times (for HW timing via wall deltas)."""
    import concourse.bacc as bacc
    import concourse.tile as tile
    import concourse.mybir as mybir
    from contextlib import ExitStack

    f32 = mybir.dt.float32
    f16 = mybir.dt.float16
    nc = bacc.Bacc("TRN2", target_bir_lowering=False, debug=False)

    xph = nc.dram_tensor("xph", [KROWS, PX], f16, kind="ExternalInput").ap()
    xpl = nc.dram_tensor("xpl", [KROWS, PX], f16, kind="ExternalInput").ap()
    phd = nc.dram_tensor("ph", [KROWS, N], f16, kind="ExternalInput").ap()
    pld = nc.dram_tensor("pl", [KROWS, N], f16, kind="ExternalInput").ap()
    vsd = nc.dram_tensor("vs16", [128, N], f16, kind="ExternalInput").ap()
    idd = nc.dram_tensor("id32", [128, 128], f32, kind="ExternalInput").ap()
    out = nc.dram_tensor("out", [BLOC, 128, H * W], f32, kind="ExternalOutput").ap()

    isge = mybir.AluOpType.is_ge

    with tile.TileContext(nc) as tc, ExitStack() as ctx:
        const = ctx.enter_context(tc.tile_pool(name="const", bufs=1))
        xbig = ctx.enter_context(tc.tile_pool(name="xbig", bufs=1))
        m8p = ctx.enter_context(tc.tile_pool(name="m8p", bufs=1))
        m16p = ctx.enter_context(tc.tile_pool(name="m16p", bufs=2))
        t8p = ctx.enter_context(tc.tile_pool(name="t8p", bufs=1))
        mkp = ctx.enter_context(tc.tile_pool(name="mkp", bufs=3))
        otp = ctx.enter_context(tc.tile_pool(name="otp", bufs=2))
        psA = ctx.enter_context(tc.tile_pool(name="psA", bufs=2, space="PSUM"))
        psSC = ctx.enter_context(tc.tile_pool(name="psSC", bufs=2, space="PSUM"))
        psOut = ctx.enter_context(tc.tile_pool(name="psOut", bufs=2, space="PSUM"))

        ph_t = const.tile([KROWS, N], f16)
        pl_t = const.tile([KROWS, N], f16)
        vs_t = const.tile([128, N], f16)
        id_t = const.tile([128, 128], f32)
        nc.sync.dma_start(ph_t[:], phd[:])
        nc.sync.dma_start(pl_t[:], pld[:])
        nc.sync.dma_start(vs_t[:], vsd[:])
        nc.sync.dma_start(id_t[:], idd[:])

        xh = xbig.tile([KROWS, PX], f16, tag="xh")
        xl = xbig.tile([KROWS, PX], f16, tag="xl")
        m8all = m8p.tile([128, 8 * NTILE], f32, tag="m8all")
        t8h16 = t8p.tile([128, NTILE], f16, tag="h16")
        t8h32 = t8p.tile([128, NTILE], f32, tag="h32")
        t8l32 = t8p.tile([128, NTILE], f32, tag="l32")
        t8s = t8p.tile([128, 2 * NTILE], f16, tag="t8s")

        loop_cm = tc.For_i(0, loop_r, 1) if loop_r else None
        if loop_cm is not None:
            loop_cm.__enter__()

        nc.sync.dma_start(xh[:], xph[:])
        nc.sync.dma_start(xl[:], xpl[:])

        # ---- phase A: score[px, n], per-pixel top-8 threshold ----
        for t in range(NTILE):
            tsl = slice(t * 128, (t + 1) * 128)
            lh = xh[0:KDIM + 1, tsl]
            ll = xl[0:KDIM + 1, tsl]
            m16 = m16p.tile([128, 16], f32, tag="m16")
            for h in range(2):
                pa = psA.tile([128, 1024], f32, tag="pa")
                for q in range(2):
                    nsl = slice(h * 1024 + q * 512, h * 1024 + (q + 1) * 512)
                    osl = slice(q * 512, (q + 1) * 512)
                    nc.tensor.matmul(pa[:, osl], lh, ph_t[0:KDIM + 1, nsl],
                                     start=True, stop=False)
                for q in range(2):
                    nsl = slice(h * 1024 + q * 512, h * 1024 + (q + 1) * 512)
                    osl = slice(q * 512, (q + 1) * 512)
                    nc.tensor.matmul(pa[:, osl], lh, pl_t[0:KDIM + 1, nsl],
                                     start=False, stop=False)
                for q in range(2):
                    nsl = slice(h * 1024 + q * 512, h * 1024 + (q + 1) * 512)
                    osl = slice(q * 512, (q + 1) * 512)
                    nc.tensor.matmul(pa[:, osl], ll, ph_t[0:KDIM + 1, nsl],
                                     start=False, stop=True)
                nc.vector.max(m16[:, h * 8:(h + 1) * 8], pa[:])
            nc.vector.max(m8all[:, t * 8:(t + 1) * 8], m16[:])

        # ---- boundary: t8 -> fp16 hi/lo, transpose, row-76 DMAs ----
        t8v = m8all[:, 7::8]                       # [128, NTILE] f32, stride 8
        nc.vector.tensor_copy(t8h16[:], t8v)       # f32 -> f16 (round)
        nc.vector.tensor_copy(t8h32[:], t8h16[:])  # f16 -> f32 (exact)
        nc.vector.tensor_sub(t8l32[:], t8v, t8h32[:])
        pt = psOut.tile([128, GPX], f32, tag="out")
        nc.tensor.transpose(pt[:, 0:NTILE], t8h32[:], id_t[:])
        nc.tensor.transpose(pt[:, NTILE:2 * NTILE], t8l32[:], id_t[:])
        nc.scalar.copy(t8s[:], pt[:, 0:2 * NTILE])  # f32 -> f16 via ACT
        nc.sync.dma_start(xh[THR_ROW:KROWS, :], t8s[:, 0:NTILE])
        nc.sync.dma_start(xl[THR_ROW:KROWS, :], t8s[:, NTILE:2 * NTILE])

        # ---- phase B: masks in [n, px] layout + values matmul ----
        for g in range(NGRP):
            b, s = divmod(g, (H * W) // GPX)
            gsl = slice(g * GPX, (g + 1) * GPX)
            po = psOut.tile([128, GPX], f32, tag="out")
            vmm = None
            for c in range(NCHUNK):
                csl = slice(c * 128, (c + 1) * 128)
                sc = psSC.tile([128, GPX], f32, tag="sc")
                nc.tensor.matmul(sc[:], ph_t[:, csl], xh[:, gsl],
                                 start=True, stop=False)
                nc.tensor.matmul(sc[:], pl_t[:, csl], xh[:, gsl],
                                 start=False, stop=False)
                nc.tensor.matmul(sc[:], ph_t[:, csl], xl[:, gsl],
                                 start=False, stop=True)
                if vmm is not None:
                    vmm()
                mk = mkp.tile([128, GPX], f16, tag="mk")
                nc.vector.tensor_scalar(mk[:], sc[:], -EPS, None, isge)

                def vmm(c=c, mk=mk):
                    nc.tensor.matmul(po[:], vs_t[:, c * 128:(c + 1) * 128],
                                     mk[:], start=(c == 0),
                                     stop=(c == NCHUNK - 1))
            vmm()
            ot = otp.tile([128, GPX], f32, tag="ot")
            nc.scalar.copy(ot[:], po[:])
            nc.sync.dma_start(out[b, :, s * GPX:(s + 1) * GPX], ot[:])

        if loop_cm is not None:
            loop_cm.__exit__(None, None, None)

    nc.compile()
    return nc


def _get_program():
    if "nc" not in _CACHE:
        _CACHE["nc"] = _build_program()
    return _CACHE["nc"]


def _im2col(x):
    """x: (B,3,64,64) f32 -> cols (B, 75, 4096) f32, k=(c,dy,dx), px=(h,w)."""
    xpad = np.pad(x, ((0, 0), (0, 0), (2, 2), (2, 2)))
    win = np.lib.stride_tricks.sliding_window_view(xpad, (KH, KW), axis=(2, 3))
    cols = np.ascontiguousarray(win.transpose(0, 1, 4, 5, 2, 3))
    return cols.reshape(x.shape[0], KDIM, H * W)


def _host_prep(x, patches, values):
    """Returns per-core in_maps list."""
    pf = patches.reshape(N, KDIM)
    bias = (-0.5 * np.sum(pf.astype(np.float64) ** 2, axis=1)).astype(np.float32)

    pfull = np.zeros((KROWS, N), np.float32)
    pfull[0:KDIM] = pf.T
    pfull[BIAS_ROW] = bias
    pfull[THR_ROW] = -1.0
    ph = pfull.astype(np.float16)
    pl = (pfull - ph.astype(np.float32)).astype(np.float16)  # row 76 -> 0

    vs16 = np.ascontiguousarray(
        (values * 0.125).reshape(NCHUNK, 128, 128).transpose(1, 0, 2).reshape(128, N)
    ).astype(np.float16)
    id32 = np.eye(128, dtype=np.float32)

    cols = _im2col(x)  # (32, 75, 4096) f32
    in_maps = []
    for i in range(NCORES):
        xfull = np.zeros((KROWS, PX), np.float32)
        xfull[0:KDIM] = np.concatenate(
            [cols[i * BLOC + j] for j in range(BLOC)], axis=1)
        xfull[BIAS_ROW] = 1.0
        xh = xfull.astype(np.float16)
        xl = (xfull - xh.astype(np.float32)).astype(np.float16)  # rows 75/76 -> 0
        in_maps.append({"xph": xh, "xpl": xl, "ph": ph, "pl": pl,
                        "vs16": vs16, "id32": id32})
    return in_maps


def kernel(x, patches, values):
    from concourse.bass_utils import run_bass_kernel_spmd

    x = np.asarray(x, dtype=np.float32)
    patches = np.asarray(patches, dtype=np.float32)
    values = np.asarray(values, dtype=np.float32)

    nc = _get_program()
    in_maps = _host_prep(x, patches, values)
    res = run_bass_kernel_spmd(nc, in_maps, list(range(NCORES)))

    out = np.empty((B, D, H, W), np.float32)
    for i in range(NCORES):
        o = res.results[i]["out"]  # (BLOC, 128, 4096)
        out[i * BLOC:(i + 1) * BLOC] = o.reshape(BLOC, D, H, W)
    return out
